# revision 10
# baseline (speedup 1.0000x reference)
"""Tensor-parallel GQA attention prefill (B=1, S=2048, D=4096, 32 q-heads /
8 kv-heads, RoPE, causal) for 8 Trainium2 NeuronCores.

Sharding: head-parallel. Core g owns q-heads 4g..4g+3 and kv-head g
(exact GQA group), computes Q/K/V projections for its heads, RoPE,
causal attention, and the partial output projection over its 512
contraction dims of wo. The host sums the 8 partial outputs.

Per-core kernel (Bass/Tile), v2 — engineered for PE saturation:

  phase 1  Q/K/V projections in seq-block-major order: for each 512-wide
           seq block, stream over all 32 contraction tiles computing all
           6 output o-tiles (4 q heads + k + v) into 6 PSUM banks. The
           activation x streams in [128,512] slices (DMA triggers split
           across the sync and vector engines) so the first matmul fires
           ~1.5us in and the PE never waits on the 16MB x load. Weights
           are resident, c-major interleaved. RoPE (J-matmul pair-swap +
           cos/sin fma) for pass sb runs in reserved slots of pass sb+1.
  phase 2  attention transposed: scoresT[k,q] per k-tile, exp on the
           scalar engine out of PSUM, unnormalized attnV accumulated in
           PSUM. Softmax denominators no longer burn a PE matmul per
           k-tile: exp tiles are accumulated elementwise in f32 on the
           gpsimd (heads 0-1) / vector (heads 2-3) engines and a single
           all-ones matmul per (block, head) lands the partition-sum
           broadcast in PSUM for the approx-reciprocal normalize.
  phase 3  output projection, interleaved into phase 2's PE stream: the
           32 matmuls per 128-row output chunk are queued when a head
           group's attention completes and drained 2-4 at a time behind
           each attnV, so the PE chews output projection while the
           scalar engine's exp chain would otherwise stall it. PSUM
           evictions rotate over scalar/vector/gpsimd; each chunk is
           stored with a single 1MB DMA.

All matmuls run in bf16 with fp32 PSUM accumulation.
"""

import sys

if "/opt/trn_rl_repo" not in sys.path:
    sys.path.insert(0, "/opt/trn_rl_repo")

from collections import deque
from contextlib import ExitStack

import numpy as np
import ml_dtypes

import concourse.bass as bass
import concourse.tile as tile
from concourse import mybir, bacc

BF16 = mybir.dt.bfloat16
F32 = mybir.dt.float32
NBF = ml_dtypes.bfloat16

S = 2048
D = 4096
HD = 128
HQ = 4                      # q heads per core
N_CORES = 8
SCALE = 1.0 / float(np.sqrt(128.0))
NEG = -1e9

NCT = D // 128              # contraction tiles over model dim
NSB = S // 512              # 512-wide seq blocks
NST = S // 128              # 128-wide seq tiles
NOV = HQ + 2                # o-tiles: 4 q heads + k + v
NO = HQ + 1                 # rotated o-tiles (q heads + k)
NJT = HQ                    # contraction j-tiles in output proj
NEB = D // 512              # output-proj e blocks


def build_nc(num_devices=N_CORES):
    nc = bacc.Bacc("TRN2", target_bir_lowering=False, debug=False,
                   num_devices=num_devices)
    xt_d = nc.dram_tensor("xt", [D, S], BF16, kind="ExternalInput")
    wt_d = nc.dram_tensor("wt", [NCT, 128, NOV * 128], BF16,
                          kind="ExternalInput")
    wot_d = nc.dram_tensor("wot", [NJT, 128, D], BF16, kind="ExternalInput")
    cos2_d = nc.dram_tensor("cos2", [128, S], F32, kind="ExternalInput")
    sin2_d = nc.dram_tensor("sin2", [128, S], F32, kind="ExternalInput")
    jt_d = nc.dram_tensor("jt", [128, 128], BF16, kind="ExternalInput")
    id_d = nc.dram_tensor("ident", [128, 128], BF16, kind="ExternalInput")
    mask_d = nc.dram_tensor("maskt", [128, 128], BF16, kind="ExternalInput")
    out_d = nc.dram_tensor("out", [S, D], BF16, kind="ExternalOutput")

    with tile.TileContext(nc) as tc, ExitStack() as outer:
        const = outer.enter_context(tc.tile_pool(name="const", bufs=1))
        qkp = outer.enter_context(tc.tile_pool(name="qkrot", bufs=1))
        vp = outer.enter_context(tc.tile_pool(name="vnat", bufs=1))
        aotp = outer.enter_context(tc.tile_pool(name="aot", bufs=1))
        wotp = outer.enter_context(tc.tile_pool(name="wotsb", bufs=1))

        jt_sb = const.tile([128, 128], BF16)
        id_sb = const.tile([128, 128], BF16)
        mask_sb = const.tile([128, 128], BF16)
        ones_sb = const.tile([128, 128], BF16)

        qk_rot = qkp.tile([128, NO * S], BF16)
        v_nat = vp.tile([128, S], BF16)
        aot = aotp.tile([128, NJT * S], BF16)
        wot_sb = wotp.tile([128, NJT * D], BF16)

        # ---------------- phase 1: projections + RoPE ----------------
        with ExitStack() as ph1:
            wsb = ph1.enter_context(tc.tile_pool(name="wsb", bufs=1))
            csp = ph1.enter_context(tc.tile_pool(name="cossin", bufs=1))
            xsp = ph1.enter_context(tc.tile_pool(name="xs", bufs=40))
            vts = ph1.enter_context(tc.tile_pool(name="vtsb", bufs=1))
            qts = ph1.enter_context(tc.tile_pool(name="qtmp", bufs=6))
            rtm = ph1.enter_context(tc.tile_pool(name="ropetmp", bufs=2))
            pps = ph1.enter_context(tc.tile_pool(name="projps", bufs=6,
                                                 space="PSUM"))
            jps = ph1.enter_context(tc.tile_pool(name="jps", bufs=2,
                                                 space="PSUM"))

            w_sb = wsb.tile([128, NCT * NOV * 128], BF16)
            cos_sb = csp.tile([128, S], F32)
            sin_sb = csp.tile([128, S], F32)
            vt_sb = vts.tile([128, S], BF16)

            x_tiles = {}

            def issue_x(sb):
                for c in range(NCT):
                    eng = nc.sync if c % 2 == 0 else nc.gpsimd
                    t = xsp.tile([128, 512], BF16, tag="xs",
                                 name=f"xs_{sb}_{c}")
                    eng.dma_start(
                        out=t,
                        in_=xt_d[c * 128:(c + 1) * 128,
                                 sb * 512:(sb + 1) * 512])
                    x_tiles[(sb, c)] = t

            # x for the first pass goes out first so the PE starts ASAP;
            # weights stream c-major on the scalar engine's queue. Later
            # x passes are issued only after the pass that frees their
            # rotating buffers has been emitted (WAR tracking).
            issue_x(0)
            nc.sync.dma_start(out=jt_sb, in_=jt_d[:])
            nc.sync.dma_start(out=id_sb, in_=id_d[:])
            nc.sync.dma_start(out=mask_sb, in_=mask_d[:])
            for c in range(NCT):
                nc.scalar.dma_start(out=w_sb[:, c * 768:(c + 1) * 768],
                                    in_=wt_d[c])
            nc.sync.dma_start(out=cos_sb, in_=cos2_d[:])
            nc.sync.dma_start(out=sin_sb, in_=sin2_d[:])
            nc.vector.memset(ones_sb, 1.0)

            def emit_rope(sb, o, qt):
                jp = jps.tile([128, 512], F32, tag="jps")
                nc.tensor.matmul(jp, jt_sb, qt, start=True, stop=True)
                t1 = rtm.tile([128, 512], F32, tag="rt")
                nc.vector.tensor_mul(
                    t1, qt, cos_sb[:, sb * 512:(sb + 1) * 512])
                nc.vector.tensor_mul(
                    jp, jp, sin_sb[:, sb * 512:(sb + 1) * 512])
                nc.vector.tensor_add(
                    qk_rot[:, o * S + sb * 512: o * S + sb * 512 + 512],
                    t1, jp)

            def emit_vtrans(t):
                tp = jps.tile([128, 128], BF16, tag="jps")
                nc.tensor.transpose(tp, vt_sb[:, t * 128:(t + 1) * 128],
                                    id_sb)
                nc.vector.tensor_copy(v_nat[:, t * 128:(t + 1) * 128], tp)

            prev_qt = [None] * NO
            for sb in range(NSB):
                psl = [pps.tile([128, 512], F32, tag="pps",
                                name=f"pps_{sb}_{o}") for o in range(NOV)]
                for c in range(NCT):
                    if sb > 0:
                        o = c - 2
                        if 0 <= o < NO:
                            emit_rope(sb - 1, o, prev_qt[o])
                        t = c - 7
                        if 0 <= t < 4:
                            emit_vtrans(4 * (sb - 1) + t)
                    for o in range(NOV):
                        nc.tensor.matmul(
                            psl[o],
                            w_sb[:, (c * NOV + o) * 128:
                                 (c * NOV + o + 1) * 128],
                            x_tiles[(sb, c)],
                            start=(c == 0), stop=(c == NCT - 1))
                if sb + 1 < NSB:
                    issue_x(sb + 1)
                if sb == 0:
                    for j in range(NJT):
                        nc.sync.dma_start(out=wot_sb[:, j * D:(j + 1) * D],
                                          in_=wot_d[j])
                for o in range(NO):
                    qt = qts.tile([128, 512], BF16, tag="qt",
                                  name=f"qt_{sb}_{o}")
                    nc.scalar.activation(
                        out=qt, in_=psl[o],
                        func=mybir.ActivationFunctionType.Copy)
                    prev_qt[o] = qt
                nc.scalar.activation(
                    out=vt_sb[:, sb * 512:(sb + 1) * 512], in_=psl[NO],
                    func=mybir.ActivationFunctionType.Copy)
            for o in range(NO):
                emit_rope(NSB - 1, o, prev_qt[o])
            for t in range(4 * (NSB - 1), NST):
                emit_vtrans(t)

        # ---------------- phase 2+3: attention with interleaved O-proj
        with ExitStack() as ph2:
            etp = ph2.enter_context(tc.tile_pool(name="expt", bufs=6))
            accfp = ph2.enter_context(tc.tile_pool(name="accf", bufs=2))
            accbp = ph2.enter_context(tc.tile_pool(name="accb", bufs=2))
            rbp = ph2.enter_context(tc.tile_pool(name="rbc", bufs=2))
            stgp = ph2.enter_context(tc.tile_pool(name="stage", bufs=2))
            spsp = ph2.enter_context(tc.tile_pool(name="sps", bufs=3,
                                                  space="PSUM"))
            outpp = ph2.enter_context(tc.tile_pool(name="outps", bufs=1,
                                                   space="PSUM"))
            opsp = ph2.enter_context(tc.tile_pool(name="ops", bufs=4,
                                                  space="PSUM"))

            ph3q = deque()
            # gpsimd cannot access PSUM: evictions go scalar/vector only,
            # while the (SBUF-only) exp accumulation lives on gpsimd.
            EVICT_ENG = [nc.scalar, nc.vector, nc.vector, nc.vector,
                         nc.vector, nc.vector, nc.vector, nc.scalar]

            def push_ph3(stc):
                stage = stgp.tile([128, D], BF16, tag="stage",
                                  name=f"stage_{stc}")
                for half in range(2):
                    psl = [opsp.tile([128, 512], F32, tag="ops",
                                     name=f"ops_{stc}_{half}_{i}")
                           for i in range(4)]
                    for j in range(NJT):
                        for i in range(4):
                            eb = half * 4 + i
                            def mm(ps=psl[i], j=j, eb=eb, stc=stc):
                                nc.tensor.matmul(
                                    ps,
                                    aot[:, j * S + stc * 128:
                                        j * S + (stc + 1) * 128],
                                    wot_sb[:, j * D + eb * 512:
                                           j * D + (eb + 1) * 512],
                                    start=(j == 0), stop=(j == NJT - 1))
                            ph3q.append((1, mm))
                            if j == NJT - 1:
                                def ev(ps=psl[i], eb=eb, stage=stage):
                                    eng = EVICT_ENG[eb]
                                    dst = stage[:, eb * 512:(eb + 1) * 512]
                                    if eng is nc.scalar:
                                        nc.scalar.activation(
                                            out=dst, in_=ps,
                                            func=mybir.ActivationFunctionType.Copy)
                                    else:
                                        eng.tensor_copy(dst, ps)
                                ph3q.append((0, ev))

                def st(stc=stc, stage=stage):
                    r0, r1 = stc * 128, (stc + 1) * 128
                    if stc == NST - 1:
                        nc.sync.dma_start(out=out_d[r0:r1, :D // 2],
                                          in_=stage[:, :D // 2])
                        nc.scalar.dma_start(out=out_d[r0:r1, D // 2:],
                                            in_=stage[:, D // 2:])
                    else:
                        nc.sync.dma_start(out=out_d[r0:r1, :], in_=stage)
                ph3q.append((0, st))

            def drain_ph3(budget):
                while ph3q and (budget is None or budget > 0
                                or ph3q[0][0] == 0):
                    cost, fn = ph3q.popleft()
                    fn()
                    if budget is not None:
                        budget -= cost

            DRAIN_K = {0: 0, 1: 4, 2: 3, 3: 2}

            for jq in range(NSB):
                nk = 4 * jq + 4
                kdrain = DRAIN_K[jq]
                for h in range(HQ):
                    acc_eng = nc.gpsimd
                    accf = accfp.tile([128, 512], F32, tag="accf")
                    accb = accbp.tile([128, 512], BF16, tag="accb")
                    outps = outpp.tile([128, 512], F32, tag="outps")
                    ets = [None] * nk

                    def stage_a(kt):
                        delta = kt - 4 * jq
                        a = max(delta, 0) * 128
                        sps = spsp.tile([128, 512], F32, tag="sps")
                        nc.tensor.matmul(
                            sps[:, a:],
                            qk_rot[:, HQ * S + kt * 128:
                                   HQ * S + (kt + 1) * 128],
                            qk_rot[:, h * S + jq * 512 + a:
                                   h * S + jq * 512 + 512],
                            start=True, stop=True)
                        if delta >= 0:
                            nc.vector.tensor_add(
                                sps[:, a:a + 128], sps[:, a:a + 128],
                                mask_sb)
                        et = etp.tile([128, 512], BF16, tag="et")
                        nc.scalar.activation(
                            out=et[:, a:], in_=sps[:, a:],
                            func=mybir.ActivationFunctionType.Exp,
                            scale=SCALE)
                        ets[kt] = (et, a)

                    def stage_b(kt):
                        et, a = ets[kt]
                        nc.tensor.matmul(
                            outps[:, a:], v_nat[:, kt * 128:(kt + 1) * 128],
                            et[:, a:],
                            start=(kt == 0), stop=(kt == nk - 1))
                        if kt == 0:
                            acc_eng.tensor_copy(accf, et)
                        else:
                            acc_eng.tensor_add(accf[:, a:], accf[:, a:],
                                               et[:, a:])
                        drain_ph3(kdrain)

                    # depth-2 software pipeline: scores/exp run one k-tile
                    # ahead of attnV so the PE never waits on the exp.
                    for kt in range(nk):
                        stage_a(kt)
                        if kt >= 1:
                            stage_b(kt - 1)
                    stage_b(nk - 1)

                    acc_eng.tensor_copy(accb, accf)
                    rps = spsp.tile([128, 512], F32, tag="sps")
                    nc.tensor.matmul(rps, ones_sb, accb, start=True,
                                     stop=True)
                    rinv = rbp.tile([128, 512], F32, tag="rinv")
                    nc.vector.reciprocal_approx_fast(out=rinv, in_=rps)
                    nc.vector.tensor_mul(
                        aot[:, h * S + jq * 512: h * S + jq * 512 + 512],
                        outps, rinv)
                    drain_ph3(2)
                if jq == NSB - 1:
                    for stc in range(4 * jq, 4 * jq + 4):
                        push_ph3(stc)
                    drain_ph3(None)
                else:
                    for stc in range(4 * jq, 4 * jq + 4):
                        push_ph3(stc)

    nc.compile()
    return nc


# ---------------------------------------------------------------------------
# host-side prep


def make_consts(cos, sin):
    """cos/sin: [S, 64] f32 -> replicated T-layout + J + identity + diag mask."""
    cos2 = np.repeat(np.ascontiguousarray(cos.T), 2, axis=0).astype(np.float32)
    sin2 = np.repeat(np.ascontiguousarray(sin.T), 2, axis=0).astype(np.float32)
    J = np.zeros((128, 128), np.float32)
    for p in range(64):
        J[2 * p, 2 * p + 1] = -1.0
        J[2 * p + 1, 2 * p] = 1.0
    jt = np.ascontiguousarray(J.T).astype(NBF)
    ident = np.eye(128, dtype=NBF)
    k_idx = np.arange(128)[:, None]
    q_idx = np.arange(128)[None, :]
    maskt = np.where(q_idx >= k_idx, 0.0, NEG).astype(np.float32)  # [k, q]
    return cos2, sin2, jt, ident, maskt.astype(NBF)


def prep_all(x, wq, wk, wv, wo, cos, sin, n_cores=N_CORES):
    x2 = np.asarray(x, np.float32).reshape(S, D)
    xt = np.ascontiguousarray(x2.T).astype(NBF)
    wq = np.asarray(wq, np.float32)
    wk = np.asarray(wk, np.float32)
    wv = np.asarray(wv, np.float32)
    wo = np.asarray(wo, np.float32)
    cos2, sin2, jt, ident, maskt = make_consts(
        np.asarray(cos, np.float32), np.asarray(sin, np.float32))
    in_maps = []
    for g in range(n_cores):
        w_cat = np.concatenate(
            [wq[g * 512:(g + 1) * 512],
             wk[g * 128:(g + 1) * 128],
             wv[g * 128:(g + 1) * 128]], axis=0)          # [768, D]
        # wt[c, p, o*128 + f] = w_cat[o*128 + f, c*128 + p]
        wt = np.ascontiguousarray(
            w_cat.reshape(NOV, 128, NCT, 128).transpose(2, 3, 0, 1)
        ).reshape(NCT, 128, NOV * 128).astype(NBF)
        wot = np.ascontiguousarray(
            wo[:, g * 512:(g + 1) * 512].T).reshape(4, 128, D).astype(NBF)
        in_maps.append({
            "xt": xt, "wt": wt, "wot": wot, "cos2": cos2, "sin2": sin2,
            "jt": jt, "ident": ident, "maskt": maskt,
        })
    return in_maps


_NC_CACHE = None


def _get_nc():
    global _NC_CACHE
    if _NC_CACHE is None:
        _NC_CACHE = build_nc()
    return _NC_CACHE


def kernel(x, wq, wk, wv, wo, cos, sin, mask, start_pos):
    # mask is the standard causal mask (start_pos=0 prefill) — the kernel
    # applies causality structurally, so neither input is shipped.
    from concourse.bass_utils import run_bass_kernel_spmd

    nc = _get_nc()
    in_maps = prep_all(x, wq, wk, wv, wo, cos, sin)
    res = run_bass_kernel_spmd(nc, in_maps, core_ids=list(range(N_CORES)))
    acc = np.zeros((S, D), np.float32)
    for r in res.results:
        acc += r["out"].astype(np.float32)
    return acc.reshape(1, S, D)


# revision 15
# speedup vs baseline: 1.4070x; 1.4070x over previous
"""Tensor-parallel GQA attention prefill (B=1, S=2048, D=4096, 32 q-heads /
8 kv-heads, RoPE, causal) for 8 Trainium2 NeuronCores.

Sharding: head-parallel. Core g owns q-heads 4g..4g+3 and kv-head g
(exact GQA group), computes Q/K/V projections for its heads, RoPE,
causal attention, and the partial output projection over its 512
contraction dims of wo. The host sums the 8 partial outputs.

Per-core kernel (Bass/Tile), v3:

  phase 1  Q/K/V projections from a resident transposed activation.
           x streams in as 64 half-row [128,1024] transfers on the sync
           queue; the first two o-tiles are computed interleaved at
           seq-half granularity so the PE's consumption rate matches the
           x arrival rate (no startup stall), then o=2..5 run o-major
           with 4-seq-block stationary reuse. Weights and the output
           projection weight stream on the scalar queue. RoPE runs in
           reserved slots of the following pass.
  phase 2  attention transposed (scoresT per k-tile, scalar-engine exp,
           attnV + all-ones denominator matmuls accumulating in PSUM —
           the denominator matmuls double as PE filler behind the exp
           chain). Normalization = approx-reciprocal + multiply on DVE.
  phase 3  output projection per 128-row chunk over 8 PSUM banks;
           evictions alternate scalar/vector, each chunk stored with a
           single 1MB DMA (the last chunk split across two queues so the
           kernel tail is short).

All matmuls run in bf16 with fp32 PSUM accumulation.
"""

import sys

if "/opt/trn_rl_repo" not in sys.path:
    sys.path.insert(0, "/opt/trn_rl_repo")

from contextlib import ExitStack

import numpy as np
import ml_dtypes

import concourse.bass as bass
import concourse.tile as tile
from concourse import mybir, bacc

BF16 = mybir.dt.bfloat16
F32 = mybir.dt.float32
NBF = ml_dtypes.bfloat16

S = 2048
D = 4096
HD = 128
HQ = 4                      # q heads per core
N_CORES = 8
SCALE = 1.0 / float(np.sqrt(128.0))
NEG = -1e9

NCT = D // 128              # contraction tiles over model dim
NSB = S // 512              # 512-wide seq blocks
NST = S // 128              # 128-wide seq tiles
NOV = HQ + 2                # o-tiles: 4 q heads + k + v
NO = HQ + 1                 # rotated o-tiles (q heads + k)
NJT = HQ                    # contraction j-tiles in output proj
NEB = D // 512              # output-proj e blocks
WCOLS = NCT * 128           # per-o weight row length


def build_nc(num_devices=N_CORES):
    nc = bacc.Bacc("TRN2", target_bir_lowering=False, debug=False,
                   num_devices=num_devices)
    xt_d = nc.dram_tensor("xt", [D, S], BF16, kind="ExternalInput")
    wt_d = nc.dram_tensor("wt", [NOV, 128, WCOLS], BF16, kind="ExternalInput")
    wot_d = nc.dram_tensor("wot", [NJT, 128, D], BF16, kind="ExternalInput")
    cos2_d = nc.dram_tensor("cos2", [128, S], F32, kind="ExternalInput")
    sin2_d = nc.dram_tensor("sin2", [128, S], F32, kind="ExternalInput")
    jt_d = nc.dram_tensor("jt", [128, 128], BF16, kind="ExternalInput")
    id_d = nc.dram_tensor("ident", [128, 128], BF16, kind="ExternalInput")
    mask_d = nc.dram_tensor("maskt", [128, 128], BF16, kind="ExternalInput")
    out_d = nc.dram_tensor("out", [S, D], BF16, kind="ExternalOutput")

    with tile.TileContext(nc) as tc, ExitStack() as outer:
        const = outer.enter_context(tc.tile_pool(name="const", bufs=1))
        qkp = outer.enter_context(tc.tile_pool(name="qkrot", bufs=1))
        vp = outer.enter_context(tc.tile_pool(name="vnat", bufs=1))

        jt_sb = const.tile([128, 128], BF16)
        id_sb = const.tile([128, 128], BF16)
        mask_sb = const.tile([128, 128], BF16)
        ones_sb = const.tile([128, 128], BF16)

        qk_rot = qkp.tile([128, NO * S], BF16)
        v_nat = vp.tile([128, S], BF16)

        # ---------------- phase 1: projections + RoPE ----------------
        with ExitStack() as ph1:
            xtp = ph1.enter_context(tc.tile_pool(name="xtp", bufs=1))
            csp = ph1.enter_context(tc.tile_pool(name="cossin", bufs=1))
            wst = ph1.enter_context(tc.tile_pool(name="wstream", bufs=2))
            vts = ph1.enter_context(tc.tile_pool(name="vtsb", bufs=1))
            qts = ph1.enter_context(tc.tile_pool(name="qtmp", bufs=5))
            rtm = ph1.enter_context(tc.tile_pool(name="ropetmp", bufs=2))
            pps = ph1.enter_context(tc.tile_pool(name="projps", bufs=8,
                                                 space="PSUM"))

            xt_sb = xtp.tile([128, NCT * S], BF16)
            cos_sb = csp.tile([128, S], F32)
            sin_sb = csp.tile([128, S], F32)
            vt_sb = vts.tile([128, S], BF16)

            # sync queue: consts, then x half-rows (first halves c-order,
            # then second halves), cos/sin slotted between.
            nc.sync.dma_start(out=jt_sb, in_=jt_d[:])
            nc.sync.dma_start(out=id_sb, in_=id_d[:])
            nc.sync.dma_start(out=mask_sb, in_=mask_d[:])
            hS = S // 2
            for c in range(16):
                nc.sync.dma_start(out=xt_sb[:, c * S: c * S + hS],
                                  in_=xt_d[c * 128:(c + 1) * 128, :hS])
            nc.sync.dma_start(out=cos_sb, in_=cos2_d[:])
            nc.sync.dma_start(out=sin_sb, in_=sin2_d[:])
            for c in range(16, NCT):
                nc.sync.dma_start(out=xt_sb[:, c * S: c * S + hS],
                                  in_=xt_d[c * 128:(c + 1) * 128, :hS])
            for c in range(NCT):
                nc.sync.dma_start(out=xt_sb[:, c * S + hS:(c + 1) * S],
                                  in_=xt_d[c * 128:(c + 1) * 128, hS:])
            nc.vector.memset(ones_sb, 1.0)

            # scalar queue: per-o projection weights then wot.
            w_tiles = []
            for o in range(NOV):
                w = wst.tile([128, WCOLS], BF16, tag="wsb", name=f"wsb_{o}")
                if o < 2:
                    # o=0/1 feed the DMA-paced opening passes: chunk them
                    # so the first c-tiles land immediately.
                    qn = WCOLS // 4
                    for qd in range(4):
                        nc.scalar.dma_start(
                            out=w[:, qd * qn:(qd + 1) * qn],
                            in_=wt_d[o, :, qd * qn:(qd + 1) * qn])
                else:
                    nc.scalar.dma_start(out=w, in_=wt_d[o])
                w_tiles.append(w)

            def emit_rope(sb, o, qt):
                jp = pps.tile([128, 512], F32, tag="ps", name=f"jp_{sb}_{o}")
                nc.tensor.matmul(jp, jt_sb, qt, start=True, stop=True)
                t1 = rtm.tile([128, 512], F32, tag="rt")
                nc.vector.tensor_mul(
                    t1, qt, cos_sb[:, sb * 512:(sb + 1) * 512])
                nc.vector.tensor_mul(
                    jp, jp, sin_sb[:, sb * 512:(sb + 1) * 512])
                nc.vector.tensor_add(
                    qk_rot[:, o * S + sb * 512: o * S + sb * 512 + 512],
                    t1, jp)

            def emit_vtrans(t):
                tp = pps.tile([128, 128], BF16, tag="ps", name=f"tp_{t}")
                nc.tensor.transpose(tp, vt_sb[:, t * 128:(t + 1) * 128],
                                    id_sb)
                nc.vector.tensor_copy(v_nat[:, t * 128:(t + 1) * 128], tp)

            # pass list: (o list, sb list); the first two passes cover
            # o=0,1 at seq-half granularity so PE consumption (~0.86us/c)
            # tracks x arrival (~0.73us/c half-rows).
            passes = [([0, 1], [0, 1]), ([0, 1], [2, 3])]
            passes += [([o], [0, 1, 2, 3]) for o in range(2, NOV)]

            # rope/vtrans work generated by earlier passes, consumed in
            # reserved slots of later passes' c-loops.
            pending = []

            def drain_pending(n):
                while pending and n > 0:
                    pending.pop(0)()
                    n -= 1

            for os_, sbs in passes:
                psl = {}
                for o in os_:
                    for sb in sbs:
                        psl[(o, sb)] = pps.tile([128, 512], F32, tag="ps",
                                                name=f"ps_{o}_{sb}")
                for c in range(NCT):
                    if c >= 2:
                        drain_pending(1)
                    for o in os_:
                        for sb in sbs:
                            nc.tensor.matmul(
                                psl[(o, sb)],
                                w_tiles[o][:, c * 128:(c + 1) * 128],
                                xt_sb[:, c * S + sb * 512:
                                      c * S + sb * 512 + 512],
                                start=(c == 0), stop=(c == NCT - 1))
                for o in os_:
                    for sb in sbs:
                        if o < NO:
                            qt = qts.tile([128, 512], BF16, tag="qt",
                                          name=f"qt_{o}_{sb}")
                            nc.scalar.activation(
                                out=qt, in_=psl[(o, sb)],
                                func=mybir.ActivationFunctionType.Copy)
                            pending.append(
                                lambda sb=sb, o=o, qt=qt: emit_rope(sb, o, qt))
                        else:
                            nc.scalar.activation(
                                out=vt_sb[:, sb * 512:(sb + 1) * 512],
                                in_=psl[(o, sb)],
                                func=mybir.ActivationFunctionType.Copy)
                            for t in range(4 * sb, 4 * sb + 4):
                                pending.append(
                                    lambda t=t: emit_vtrans(t))
            drain_pending(len(pending))

        # ---------------- phase 2: attention ----------------
        aotp = outer.enter_context(tc.tile_pool(name="aot", bufs=1))
        aot = aotp.tile([128, NJT * S], BF16)
        wotp = outer.enter_context(tc.tile_pool(name="wotsb", bufs=1))
        wot_sb = wotp.tile([128, NJT * D], BF16)
        # wot streams on the scalar queue while attention's first block
        # runs; phase 3 first reads it well after it lands.
        for j in range(NJT):
            nc.scalar.dma_start(out=wot_sb[:, j * D:(j + 1) * D],
                                in_=wot_d[j])

        with ExitStack() as ph2:
            etp = ph2.enter_context(tc.tile_pool(name="expt", bufs=6))
            rbp = ph2.enter_context(tc.tile_pool(name="rbc", bufs=2))
            spsp = ph2.enter_context(tc.tile_pool(name="sps", bufs=4,
                                                  space="PSUM"))
            outpp = ph2.enter_context(tc.tile_pool(name="outps", bufs=2,
                                                   space="PSUM"))
            rpsp = ph2.enter_context(tc.tile_pool(name="rps", bufs=2,
                                                  space="PSUM"))

            for jq in range(NSB):
                nk = 4 * jq + 4       # causal: k-tiles 0..4jq+3
                for h in range(HQ):
                    outps = outpp.tile([128, 512], F32, tag="outps")
                    rps = rpsp.tile([128, 512], F32, tag="rps")
                    for kt in range(nk):
                        delta = kt - 4 * jq
                        a = max(delta, 0) * 128   # live q range [a, 512)
                        sps = spsp.tile([128, 512], F32, tag="sps")
                        nc.tensor.matmul(
                            sps[:, a:],
                            qk_rot[:, HQ * S + kt * 128:
                                   HQ * S + (kt + 1) * 128],
                            qk_rot[:, h * S + jq * 512 + a:
                                   h * S + jq * 512 + 512],
                            start=True, stop=True)
                        if delta >= 0:
                            nc.vector.tensor_add(
                                sps[:, a:a + 128], sps[:, a:a + 128],
                                mask_sb)
                        et = etp.tile([128, 512], BF16, tag="et")
                        nc.scalar.activation(
                            out=et[:, a:], in_=sps[:, a:],
                            func=mybir.ActivationFunctionType.Exp,
                            scale=SCALE)
                        nc.tensor.matmul(
                            outps[:, a:], v_nat[:, kt * 128:(kt + 1) * 128],
                            et[:, a:],
                            start=(kt == 0), stop=(kt == nk - 1))
                        # all-ones stationary -> denominators land
                        # partition-broadcast in PSUM
                        nc.tensor.matmul(
                            rps[:, a:], ones_sb, et[:, a:],
                            start=(kt == 0), stop=(kt == nk - 1))
                    rinv = rbp.tile([128, 512], F32, tag="rinv")
                    nc.vector.reciprocal_approx_fast(out=rinv, in_=rps)
                    nc.vector.tensor_mul(
                        aot[:, h * S + jq * 512: h * S + jq * 512 + 512],
                        outps, rinv)

        # ---------------- phase 3: output projection ----------------
        with ExitStack() as ph3:
            stgp = ph3.enter_context(tc.tile_pool(name="stage", bufs=2))
            opsp = ph3.enter_context(tc.tile_pool(name="ops", bufs=8,
                                                  space="PSUM"))

            for stc in range(NST):
                stage = stgp.tile([128, D], BF16, tag="stage",
                                  name=f"stage_{stc}")
                psl = [opsp.tile([128, 512], F32, tag="ops",
                                 name=f"ops_{stc}_{i}")
                       for i in range(NEB)]
                for j in range(NJT):
                    for eb in range(NEB):
                        nc.tensor.matmul(
                            psl[eb],
                            aot[:, j * S + stc * 128: j * S + (stc + 1) * 128],
                            wot_sb[:, j * D + eb * 512: j * D + eb * 512 + 512],
                            start=(j == 0), stop=(j == NJT - 1))
                for eb in range(NEB):
                    dst = stage[:, eb * 512:(eb + 1) * 512]
                    if eb % 2 == 0:
                        nc.scalar.activation(
                            out=dst, in_=psl[eb],
                            func=mybir.ActivationFunctionType.Copy)
                    else:
                        nc.vector.tensor_copy(dst, psl[eb])
                r0, r1 = stc * 128, (stc + 1) * 128
                if stc == NST - 1:
                    nc.sync.dma_start(out=out_d[r0:r1, :D // 2],
                                      in_=stage[:, :D // 2])
                    nc.scalar.dma_start(out=out_d[r0:r1, D // 2:],
                                        in_=stage[:, D // 2:])
                else:
                    nc.sync.dma_start(out=out_d[r0:r1, :], in_=stage)

    nc.compile()
    return nc


# ---------------------------------------------------------------------------
# host-side prep


def make_consts(cos, sin):
    """cos/sin: [S, 64] f32 -> replicated T-layout + J + identity + diag mask."""
    cos2 = np.repeat(np.ascontiguousarray(cos.T), 2, axis=0).astype(np.float32)
    sin2 = np.repeat(np.ascontiguousarray(sin.T), 2, axis=0).astype(np.float32)
    J = np.zeros((128, 128), np.float32)
    for p in range(64):
        J[2 * p, 2 * p + 1] = -1.0
        J[2 * p + 1, 2 * p] = 1.0
    jt = np.ascontiguousarray(J.T).astype(NBF)
    ident = np.eye(128, dtype=NBF)
    k_idx = np.arange(128)[:, None]
    q_idx = np.arange(128)[None, :]
    maskt = np.where(q_idx >= k_idx, 0.0, NEG).astype(np.float32)  # [k, q]
    return cos2, sin2, jt, ident, maskt.astype(NBF)


def prep_all(x, wq, wk, wv, wo, cos, sin, n_cores=N_CORES):
    x2 = np.asarray(x, np.float32).reshape(S, D)
    xt = np.ascontiguousarray(x2.T).astype(NBF)
    wq = np.asarray(wq, np.float32)
    wk = np.asarray(wk, np.float32)
    wv = np.asarray(wv, np.float32)
    wo = np.asarray(wo, np.float32)
    cos2, sin2, jt, ident, maskt = make_consts(
        np.asarray(cos, np.float32), np.asarray(sin, np.float32))
    in_maps = []
    for g in range(n_cores):
        w_cat = np.concatenate(
            [wq[g * 512:(g + 1) * 512],
             wk[g * 128:(g + 1) * 128],
             wv[g * 128:(g + 1) * 128]], axis=0)          # [768, D]
        # wt[o, p, c*128 + f] = w_cat[o*128 + f, c*128 + p]
        wt = np.ascontiguousarray(
            w_cat.reshape(NOV, 128, NCT, 128).transpose(0, 3, 2, 1)
        ).reshape(NOV, 128, NCT * 128).astype(NBF)
        wot = np.ascontiguousarray(
            wo[:, g * 512:(g + 1) * 512].T).reshape(4, 128, D).astype(NBF)
        in_maps.append({
            "xt": xt, "wt": wt, "wot": wot, "cos2": cos2, "sin2": sin2,
            "jt": jt, "ident": ident, "maskt": maskt,
        })
    return in_maps


_NC_CACHE = None


def _get_nc():
    global _NC_CACHE
    if _NC_CACHE is None:
        _NC_CACHE = build_nc()
    return _NC_CACHE


def kernel(x, wq, wk, wv, wo, cos, sin, mask, start_pos):
    # mask is the standard causal mask (start_pos=0 prefill) — the kernel
    # applies causality structurally, so neither input is shipped.
    from concourse.bass_utils import run_bass_kernel_spmd

    nc = _get_nc()
    in_maps = prep_all(x, wq, wk, wv, wo, cos, sin)
    res = run_bass_kernel_spmd(nc, in_maps, core_ids=list(range(N_CORES)))
    acc = np.zeros((S, D), np.float32)
    for r in res.results:
        acc += r["out"].astype(np.float32)
    return acc.reshape(1, S, D)


# revision 17
# speedup vs baseline: 1.4219x; 1.0106x over previous
"""Tensor-parallel GQA attention prefill (B=1, S=2048, D=4096, 32 q-heads /
8 kv-heads, RoPE, causal) for 8 Trainium2 NeuronCores.

Sharding: head-parallel. Core g owns q-heads 4g..4g+3 and kv-head g
(exact GQA group), computes Q/K/V projections for its heads, RoPE,
causal attention, and the partial output projection over its 512
contraction dims of wo. The host sums the 8 partial outputs.

Per-core kernel (Bass/Tile), v3:

  phase 1  Q/K/V projections from a resident transposed activation.
           x streams in as 64 half-row [128,1024] transfers on the sync
           queue; the first two o-tiles are computed interleaved at
           seq-half granularity so the PE's consumption rate matches the
           x arrival rate (no startup stall), then o=2..5 run o-major
           with 4-seq-block stationary reuse. Weights and the output
           projection weight stream on the scalar queue. RoPE runs in
           reserved slots of the following pass.
  phase 2  attention transposed (scoresT per k-tile, scalar-engine exp,
           attnV + all-ones denominator matmuls accumulating in PSUM —
           the denominator matmuls double as PE filler behind the exp
           chain). Normalization = approx-reciprocal + multiply on DVE.
  phase 3  output projection per 128-row chunk over 8 PSUM banks;
           evictions alternate scalar/vector, each chunk stored with a
           single 1MB DMA (the last chunk split across two queues so the
           kernel tail is short).

All matmuls run in bf16 with fp32 PSUM accumulation.
"""

import sys

if "/opt/trn_rl_repo" not in sys.path:
    sys.path.insert(0, "/opt/trn_rl_repo")

from contextlib import ExitStack

import numpy as np
import ml_dtypes

import concourse.bass as bass
import concourse.tile as tile
from concourse import mybir, bacc

BF16 = mybir.dt.bfloat16
F32 = mybir.dt.float32
NBF = ml_dtypes.bfloat16

S = 2048
D = 4096
HD = 128
HQ = 4                      # q heads per core
N_CORES = 8
SCALE = 1.0 / float(np.sqrt(128.0))
NEG = -1e9

NCT = D // 128              # contraction tiles over model dim
NSB = S // 512              # 512-wide seq blocks
NST = S // 128              # 128-wide seq tiles
NOV = HQ + 2                # o-tiles: 4 q heads + k + v
NO = HQ + 1                 # rotated o-tiles (q heads + k)
NJT = HQ                    # contraction j-tiles in output proj
NEB = D // 512              # output-proj e blocks
WCOLS = NCT * 128           # per-o weight row length


def build_nc(num_devices=N_CORES):
    nc = bacc.Bacc("TRN2", target_bir_lowering=False, debug=False,
                   num_devices=num_devices)
    xt_d = nc.dram_tensor("xt", [D, S], BF16, kind="ExternalInput")
    wt_d = nc.dram_tensor("wt", [NOV, 128, WCOLS], BF16, kind="ExternalInput")
    wot_d = nc.dram_tensor("wot", [NJT, 128, D], BF16, kind="ExternalInput")
    cos2_d = nc.dram_tensor("cos2", [128, S], F32, kind="ExternalInput")
    sin2_d = nc.dram_tensor("sin2", [128, S], F32, kind="ExternalInput")
    jt_d = nc.dram_tensor("jt", [128, 128], BF16, kind="ExternalInput")
    id_d = nc.dram_tensor("ident", [128, 128], BF16, kind="ExternalInput")
    mask_d = nc.dram_tensor("maskt", [128, 128], BF16, kind="ExternalInput")
    out_d = nc.dram_tensor("out", [S, D], BF16, kind="ExternalOutput")

    with tile.TileContext(nc) as tc, ExitStack() as outer:
        const = outer.enter_context(tc.tile_pool(name="const", bufs=1))
        qkp = outer.enter_context(tc.tile_pool(name="qkrot", bufs=1))
        vp = outer.enter_context(tc.tile_pool(name="vnat", bufs=1))

        jt_sb = const.tile([128, 128], BF16)
        id_sb = const.tile([128, 128], BF16)
        mask_sb = const.tile([128, 128], BF16)
        ones_sb = const.tile([128, 128], BF16)

        qk_rot = qkp.tile([128, NO * S], BF16)
        v_nat = vp.tile([128, S], BF16)

        # ---------------- phase 1: projections + RoPE ----------------
        with ExitStack() as ph1:
            xtp = ph1.enter_context(tc.tile_pool(name="xtp", bufs=1))
            csp = ph1.enter_context(tc.tile_pool(name="cossin", bufs=1))
            wst = ph1.enter_context(tc.tile_pool(name="wstream", bufs=2))
            vts = ph1.enter_context(tc.tile_pool(name="vtsb", bufs=1))
            qts = ph1.enter_context(tc.tile_pool(name="qtmp", bufs=5))
            rtm = ph1.enter_context(tc.tile_pool(name="ropetmp", bufs=2))
            pps = ph1.enter_context(tc.tile_pool(name="projps", bufs=8,
                                                 space="PSUM"))

            xt_sb = xtp.tile([128, NCT * S], BF16)
            cos_sb = csp.tile([128, S], F32)
            sin_sb = csp.tile([128, S], F32)
            vt_sb = vts.tile([128, S], BF16)

            # sync queue: consts, then x half-rows (first halves c-order,
            # then second halves), cos/sin slotted between.
            nc.sync.dma_start(out=jt_sb, in_=jt_d[:])
            nc.sync.dma_start(out=id_sb, in_=id_d[:])
            nc.sync.dma_start(out=mask_sb, in_=mask_d[:])
            hS = S // 2
            for c in range(NCT):
                nc.sync.dma_start(out=xt_sb[:, c * S: c * S + hS],
                                  in_=xt_d[c * 128:(c + 1) * 128, :hS])
            for c in range(8):
                nc.sync.dma_start(out=xt_sb[:, c * S + hS:(c + 1) * S],
                                  in_=xt_d[c * 128:(c + 1) * 128, hS:])
            nc.sync.dma_start(out=cos_sb, in_=cos2_d[:])
            nc.sync.dma_start(out=sin_sb, in_=sin2_d[:])
            for c in range(8, NCT):
                nc.sync.dma_start(out=xt_sb[:, c * S + hS:(c + 1) * S],
                                  in_=xt_d[c * 128:(c + 1) * 128, hS:])
            nc.vector.memset(ones_sb, 1.0)

            # scalar queue: projection weights. o=0 and o=1 both feed the
            # opening pass from c=0, so their chunks interleave.
            w_tiles = [wst.tile([128, WCOLS], BF16, tag="wsb",
                                name=f"wsb_{o}") for o in range(NOV)]
            qn = WCOLS // 8
            for qd in range(8):
                for o in range(2):
                    nc.scalar.dma_start(
                        out=w_tiles[o][:, qd * qn:(qd + 1) * qn],
                        in_=wt_d[o, :, qd * qn:(qd + 1) * qn])
            for o in range(2, NOV):
                nc.scalar.dma_start(out=w_tiles[o], in_=wt_d[o])

            def emit_rope(sb, o, qt):
                jp = pps.tile([128, 512], F32, tag="ps", name=f"jp_{sb}_{o}")
                nc.tensor.matmul(jp, jt_sb, qt, start=True, stop=True)
                t1 = rtm.tile([128, 512], F32, tag="rt")
                nc.vector.tensor_mul(
                    t1, qt, cos_sb[:, sb * 512:(sb + 1) * 512])
                nc.vector.tensor_mul(
                    jp, jp, sin_sb[:, sb * 512:(sb + 1) * 512])
                nc.vector.tensor_add(
                    qk_rot[:, o * S + sb * 512: o * S + sb * 512 + 512],
                    t1, jp)

            def emit_vtrans(t):
                tp = pps.tile([128, 128], BF16, tag="ps", name=f"tp_{t}")
                nc.tensor.transpose(tp, vt_sb[:, t * 128:(t + 1) * 128],
                                    id_sb)
                nc.vector.tensor_copy(v_nat[:, t * 128:(t + 1) * 128], tp)

            # pass list: (o list, sb list); the first two passes cover
            # o=0,1 at seq-half granularity so PE consumption (~0.86us/c)
            # tracks x arrival (~0.73us/c half-rows).
            passes = [([0, 1], [0, 1]), ([0, 1], [2, 3])]
            passes += [([o], [0, 1, 2, 3]) for o in range(2, NOV)]

            # rope/vtrans work generated by earlier passes, consumed in
            # reserved slots of later passes' c-loops.
            pending = []

            def drain_pending(n):
                while pending and n > 0:
                    pending.pop(0)()
                    n -= 1

            for os_, sbs in passes:
                psl = {}
                for o in os_:
                    for sb in sbs:
                        psl[(o, sb)] = pps.tile([128, 512], F32, tag="ps",
                                                name=f"ps_{o}_{sb}")
                for c in range(NCT):
                    if c >= 2:
                        drain_pending(1)
                    for o in os_:
                        for sb in sbs:
                            nc.tensor.matmul(
                                psl[(o, sb)],
                                w_tiles[o][:, c * 128:(c + 1) * 128],
                                xt_sb[:, c * S + sb * 512:
                                      c * S + sb * 512 + 512],
                                start=(c == 0), stop=(c == NCT - 1))
                for o in os_:
                    for sb in sbs:
                        if o < NO:
                            qt = qts.tile([128, 512], BF16, tag="qt",
                                          name=f"qt_{o}_{sb}")
                            nc.scalar.activation(
                                out=qt, in_=psl[(o, sb)],
                                func=mybir.ActivationFunctionType.Copy)
                            pending.append(
                                lambda sb=sb, o=o, qt=qt: emit_rope(sb, o, qt))
                        else:
                            nc.scalar.activation(
                                out=vt_sb[:, sb * 512:(sb + 1) * 512],
                                in_=psl[(o, sb)],
                                func=mybir.ActivationFunctionType.Copy)
                            for t in range(4 * sb, 4 * sb + 4):
                                pending.append(
                                    lambda t=t: emit_vtrans(t))
            drain_pending(len(pending))

        # ---------------- phase 2: attention ----------------
        aotp = outer.enter_context(tc.tile_pool(name="aot", bufs=1))
        aot = aotp.tile([128, NJT * S], BF16)
        wotp = outer.enter_context(tc.tile_pool(name="wotsb", bufs=1))
        wot_sb = wotp.tile([128, NJT * D], BF16)
        # wot streams on the scalar queue while attention's first block
        # runs; phase 3 first reads it well after it lands.
        for j in range(NJT):
            nc.scalar.dma_start(out=wot_sb[:, j * D:(j + 1) * D],
                                in_=wot_d[j])

        with ExitStack() as ph2:
            etp = ph2.enter_context(tc.tile_pool(name="expt", bufs=6))
            rbp = ph2.enter_context(tc.tile_pool(name="rbc", bufs=2))
            spsp = ph2.enter_context(tc.tile_pool(name="sps", bufs=4,
                                                  space="PSUM"))
            outpp = ph2.enter_context(tc.tile_pool(name="outps", bufs=2,
                                                   space="PSUM"))
            rpsp = ph2.enter_context(tc.tile_pool(name="rps", bufs=2,
                                                  space="PSUM"))

            for jq in range(NSB):
                nk = 4 * jq + 4       # causal: k-tiles 0..4jq+3
                for h in range(HQ):
                    outps = outpp.tile([128, 512], F32, tag="outps")
                    rps = rpsp.tile([128, 512], F32, tag="rps")
                    for kt in range(nk):
                        delta = kt - 4 * jq
                        a = max(delta, 0) * 128   # live q range [a, 512)
                        sps = spsp.tile([128, 512], F32, tag="sps")
                        nc.tensor.matmul(
                            sps[:, a:],
                            qk_rot[:, HQ * S + kt * 128:
                                   HQ * S + (kt + 1) * 128],
                            qk_rot[:, h * S + jq * 512 + a:
                                   h * S + jq * 512 + 512],
                            start=True, stop=True)
                        if delta >= 0:
                            nc.vector.tensor_add(
                                sps[:, a:a + 128], sps[:, a:a + 128],
                                mask_sb)
                        et = etp.tile([128, 512], BF16, tag="et")
                        nc.scalar.activation(
                            out=et[:, a:], in_=sps[:, a:],
                            func=mybir.ActivationFunctionType.Exp,
                            scale=SCALE)
                        nc.tensor.matmul(
                            outps[:, a:], v_nat[:, kt * 128:(kt + 1) * 128],
                            et[:, a:],
                            start=(kt == 0), stop=(kt == nk - 1))
                        # all-ones stationary -> denominators land
                        # partition-broadcast in PSUM
                        nc.tensor.matmul(
                            rps[:, a:], ones_sb, et[:, a:],
                            start=(kt == 0), stop=(kt == nk - 1))
                    rinv = rbp.tile([128, 512], F32, tag="rinv")
                    nc.vector.reciprocal_approx_fast(out=rinv, in_=rps)
                    nc.vector.tensor_mul(
                        aot[:, h * S + jq * 512: h * S + jq * 512 + 512],
                        outps, rinv)

        # ---------------- phase 3: output projection ----------------
        with ExitStack() as ph3:
            stgp = ph3.enter_context(tc.tile_pool(name="stage", bufs=2))
            opsp = ph3.enter_context(tc.tile_pool(name="ops", bufs=8,
                                                  space="PSUM"))

            for stc in range(NST):
                stage = stgp.tile([128, D], BF16, tag="stage",
                                  name=f"stage_{stc}")
                psl = [opsp.tile([128, 512], F32, tag="ops",
                                 name=f"ops_{stc}_{i}")
                       for i in range(NEB)]
                for j in range(NJT):
                    for eb in range(NEB):
                        nc.tensor.matmul(
                            psl[eb],
                            aot[:, j * S + stc * 128: j * S + (stc + 1) * 128],
                            wot_sb[:, j * D + eb * 512: j * D + eb * 512 + 512],
                            start=(j == 0), stop=(j == NJT - 1))
                for eb in range(NEB):
                    dst = stage[:, eb * 512:(eb + 1) * 512]
                    if eb % 2 == 0:
                        nc.scalar.activation(
                            out=dst, in_=psl[eb],
                            func=mybir.ActivationFunctionType.Copy)
                    else:
                        nc.vector.tensor_copy(dst, psl[eb])
                r0, r1 = stc * 128, (stc + 1) * 128
                if stc == NST - 1:
                    nc.sync.dma_start(out=out_d[r0:r1, :D // 2],
                                      in_=stage[:, :D // 2])
                    nc.scalar.dma_start(out=out_d[r0:r1, D // 2:],
                                        in_=stage[:, D // 2:])
                else:
                    nc.sync.dma_start(out=out_d[r0:r1, :], in_=stage)

    nc.compile()
    return nc


# ---------------------------------------------------------------------------
# host-side prep


def make_consts(cos, sin):
    """cos/sin: [S, 64] f32 -> replicated T-layout + J + identity + diag mask."""
    cos2 = np.repeat(np.ascontiguousarray(cos.T), 2, axis=0).astype(np.float32)
    sin2 = np.repeat(np.ascontiguousarray(sin.T), 2, axis=0).astype(np.float32)
    J = np.zeros((128, 128), np.float32)
    for p in range(64):
        J[2 * p, 2 * p + 1] = -1.0
        J[2 * p + 1, 2 * p] = 1.0
    jt = np.ascontiguousarray(J.T).astype(NBF)
    ident = np.eye(128, dtype=NBF)
    k_idx = np.arange(128)[:, None]
    q_idx = np.arange(128)[None, :]
    maskt = np.where(q_idx >= k_idx, 0.0, NEG).astype(np.float32)  # [k, q]
    return cos2, sin2, jt, ident, maskt.astype(NBF)


def prep_all(x, wq, wk, wv, wo, cos, sin, n_cores=N_CORES):
    x2 = np.asarray(x, np.float32).reshape(S, D)
    xt = np.ascontiguousarray(x2.T).astype(NBF)
    wq = np.asarray(wq, np.float32)
    wk = np.asarray(wk, np.float32)
    wv = np.asarray(wv, np.float32)
    wo = np.asarray(wo, np.float32)
    cos2, sin2, jt, ident, maskt = make_consts(
        np.asarray(cos, np.float32), np.asarray(sin, np.float32))
    in_maps = []
    for g in range(n_cores):
        w_cat = np.concatenate(
            [wq[g * 512:(g + 1) * 512],
             wk[g * 128:(g + 1) * 128],
             wv[g * 128:(g + 1) * 128]], axis=0)          # [768, D]
        # wt[o, p, c*128 + f] = w_cat[o*128 + f, c*128 + p]
        wt = np.ascontiguousarray(
            w_cat.reshape(NOV, 128, NCT, 128).transpose(0, 3, 2, 1)
        ).reshape(NOV, 128, NCT * 128).astype(NBF)
        wot = np.ascontiguousarray(
            wo[:, g * 512:(g + 1) * 512].T).reshape(4, 128, D).astype(NBF)
        in_maps.append({
            "xt": xt, "wt": wt, "wot": wot, "cos2": cos2, "sin2": sin2,
            "jt": jt, "ident": ident, "maskt": maskt,
        })
    return in_maps


_NC_CACHE = None


def _get_nc():
    global _NC_CACHE
    if _NC_CACHE is None:
        _NC_CACHE = build_nc()
    return _NC_CACHE


def kernel(x, wq, wk, wv, wo, cos, sin, mask, start_pos):
    # mask is the standard causal mask (start_pos=0 prefill) — the kernel
    # applies causality structurally, so neither input is shipped.
    from concourse.bass_utils import run_bass_kernel_spmd

    nc = _get_nc()
    in_maps = prep_all(x, wq, wk, wv, wo, cos, sin)
    res = run_bass_kernel_spmd(nc, in_maps, core_ids=list(range(N_CORES)))
    acc = np.zeros((S, D), np.float32)
    for r in res.results:
        acc += r["out"].astype(np.float32)
    return acc.reshape(1, S, D)


# revision 18
# speedup vs baseline: 1.4760x; 1.0381x over previous
"""Tensor-parallel GQA attention prefill (B=1, S=2048, D=4096, 32 q-heads /
8 kv-heads, RoPE, causal) for 8 Trainium2 NeuronCores.

Sharding: head-parallel. Core g owns q-heads 4g..4g+3 and kv-head g
(exact GQA group), computes Q/K/V projections for its heads, RoPE,
causal attention, and the partial output projection over its 512
contraction dims of wo. The host sums the 8 partial outputs.

Per-core kernel (Bass/Tile), v3:

  phase 1  Q/K/V projections from a resident transposed activation.
           x streams in as 64 half-row [128,1024] transfers on the sync
           queue; the first two o-tiles are computed interleaved at
           seq-half granularity so the PE's consumption rate matches the
           x arrival rate (no startup stall), then o=2..5 run o-major
           with 4-seq-block stationary reuse. Weights and the output
           projection weight stream on the scalar queue. RoPE runs in
           reserved slots of the following pass.
  phase 2  attention transposed (scoresT per k-tile, scalar-engine exp,
           attnV + all-ones denominator matmuls accumulating in PSUM —
           the denominator matmuls double as PE filler behind the exp
           chain). Normalization = approx-reciprocal + multiply on DVE.
  phase 3  output projection per 128-row chunk over 8 PSUM banks;
           evictions alternate scalar/vector, each chunk stored with a
           single 1MB DMA (the last chunk split across two queues so the
           kernel tail is short).

All matmuls run in bf16 with fp32 PSUM accumulation.
"""

import sys

if "/opt/trn_rl_repo" not in sys.path:
    sys.path.insert(0, "/opt/trn_rl_repo")

from contextlib import ExitStack

import numpy as np
import ml_dtypes

import concourse.bass as bass
import concourse.tile as tile
from concourse import mybir, bacc

BF16 = mybir.dt.bfloat16
F32 = mybir.dt.float32
NBF = ml_dtypes.bfloat16

S = 2048
D = 4096
HD = 128
HQ = 4                      # q heads per core
N_CORES = 8
SCALE = 1.0 / float(np.sqrt(128.0))
NEG = -1e9

NCT = D // 128              # contraction tiles over model dim
NSB = S // 512              # 512-wide seq blocks
NST = S // 128              # 128-wide seq tiles
NOV = HQ + 2                # o-tiles: 4 q heads + k + v
NO = HQ + 1                 # rotated o-tiles (q heads + k)
NJT = HQ                    # contraction j-tiles in output proj
NEB = D // 512              # output-proj e blocks
WCOLS = NCT * 128           # per-o weight row length


def build_nc(num_devices=N_CORES):
    nc = bacc.Bacc("TRN2", target_bir_lowering=False, debug=False,
                   num_devices=num_devices)
    xt_d = nc.dram_tensor("xt", [D, S], BF16, kind="ExternalInput")
    wt_d = nc.dram_tensor("wt", [NOV, 128, WCOLS], BF16, kind="ExternalInput")
    wot_d = nc.dram_tensor("wot", [NJT, 128, D], BF16, kind="ExternalInput")
    cos2_d = nc.dram_tensor("cos2", [128, S], F32, kind="ExternalInput")
    sin2_d = nc.dram_tensor("sin2", [128, S], F32, kind="ExternalInput")
    jt_d = nc.dram_tensor("jt", [128, 128], BF16, kind="ExternalInput")
    id_d = nc.dram_tensor("ident", [128, 128], BF16, kind="ExternalInput")
    mask_d = nc.dram_tensor("maskt", [128, 128], BF16, kind="ExternalInput")
    out_d = nc.dram_tensor("out", [S, D], BF16, kind="ExternalOutput")

    with tile.TileContext(nc) as tc, ExitStack() as outer:
        const = outer.enter_context(tc.tile_pool(name="const", bufs=1))
        qkp = outer.enter_context(tc.tile_pool(name="qkrot", bufs=1))
        vp = outer.enter_context(tc.tile_pool(name="vnat", bufs=1))

        jt_sb = const.tile([128, 128], BF16)
        id_sb = const.tile([128, 128], BF16)
        mask_sb = const.tile([128, 128], BF16)
        ones_sb = const.tile([128, 128], BF16)

        qk_rot = qkp.tile([128, NO * S], BF16)
        v_nat = vp.tile([128, S], BF16)

        # ---------------- phase 1: projections + RoPE ----------------
        with ExitStack() as ph1:
            xtp = ph1.enter_context(tc.tile_pool(name="xtp", bufs=1))
            csp = ph1.enter_context(tc.tile_pool(name="cossin", bufs=1))
            wst = ph1.enter_context(tc.tile_pool(name="wstream", bufs=2))
            vts = ph1.enter_context(tc.tile_pool(name="vtsb", bufs=1))
            qts = ph1.enter_context(tc.tile_pool(name="qtmp", bufs=5))
            rtm = ph1.enter_context(tc.tile_pool(name="ropetmp", bufs=2))
            pps = ph1.enter_context(tc.tile_pool(name="projps", bufs=8,
                                                 space="PSUM"))

            xt_sb = xtp.tile([128, NCT * S], BF16)
            cos_sb = csp.tile([128, S], F32)
            sin_sb = csp.tile([128, S], F32)
            vt_sb = vts.tile([128, S], BF16)

            # sync queue: consts, then x half-rows (first halves c-order,
            # then second halves), cos/sin slotted between.
            nc.sync.dma_start(out=jt_sb, in_=jt_d[:])
            nc.sync.dma_start(out=id_sb, in_=id_d[:])
            nc.sync.dma_start(out=mask_sb, in_=mask_d[:])
            hS = S // 2
            for c in range(NCT):
                nc.sync.dma_start(out=xt_sb[:, c * S: c * S + hS],
                                  in_=xt_d[c * 128:(c + 1) * 128, :hS])
            for c in range(8):
                nc.sync.dma_start(out=xt_sb[:, c * S + hS:(c + 1) * S],
                                  in_=xt_d[c * 128:(c + 1) * 128, hS:])
            nc.sync.dma_start(out=cos_sb, in_=cos2_d[:])
            nc.sync.dma_start(out=sin_sb, in_=sin2_d[:])
            for c in range(8, NCT):
                nc.sync.dma_start(out=xt_sb[:, c * S + hS:(c + 1) * S],
                                  in_=xt_d[c * 128:(c + 1) * 128, hS:])
            nc.vector.memset(ones_sb, 1.0)

            # scalar queue: projection weights. o=0 and o=1 both feed the
            # opening pass from c=0, so their chunks interleave.
            w_tiles = [wst.tile([128, WCOLS], BF16, tag="wsb",
                                name=f"wsb_{o}") for o in range(NOV)]
            qn = WCOLS // 8
            for qd in range(8):
                for o in range(2):
                    nc.scalar.dma_start(
                        out=w_tiles[o][:, qd * qn:(qd + 1) * qn],
                        in_=wt_d[o, :, qd * qn:(qd + 1) * qn])
            for o in range(2, NOV):
                nc.scalar.dma_start(out=w_tiles[o], in_=wt_d[o])

            def emit_rope(sb, o, qt):
                jp = pps.tile([128, 512], F32, tag="ps", name=f"jp_{sb}_{o}")
                nc.tensor.matmul(jp, jt_sb, qt, start=True, stop=True)
                t1 = rtm.tile([128, 512], F32, tag="rt")
                nc.vector.tensor_mul(
                    t1, qt, cos_sb[:, sb * 512:(sb + 1) * 512])
                nc.vector.tensor_mul(
                    jp, jp, sin_sb[:, sb * 512:(sb + 1) * 512])
                nc.vector.tensor_add(
                    qk_rot[:, o * S + sb * 512: o * S + sb * 512 + 512],
                    t1, jp)

            def emit_vtrans(t):
                tp = pps.tile([128, 128], BF16, tag="ps", name=f"tp_{t}")
                nc.tensor.transpose(tp, vt_sb[:, t * 128:(t + 1) * 128],
                                    id_sb)
                nc.vector.tensor_copy(v_nat[:, t * 128:(t + 1) * 128], tp)

            # pass list: (o list, sb list); the first two passes cover
            # o=0,1 at seq-half granularity so PE consumption (~0.86us/c)
            # tracks x arrival (~0.73us/c half-rows).
            passes = [([0, 1], [0, 1]), ([0, 1], [2, 3])]
            passes += [([o], [0, 1, 2, 3]) for o in range(2, NOV)]

            # rope/vtrans work generated by earlier passes, consumed in
            # reserved slots of later passes' c-loops.
            pending = []

            def drain_pending(n):
                while pending and n > 0:
                    pending.pop(0)()
                    n -= 1

            for os_, sbs in passes:
                psl = {}
                for o in os_:
                    for sb in sbs:
                        psl[(o, sb)] = pps.tile([128, 512], F32, tag="ps",
                                                name=f"ps_{o}_{sb}")
                for c in range(NCT):
                    if c >= 2:
                        drain_pending(1)
                    for o in os_:
                        for sb in sbs:
                            nc.tensor.matmul(
                                psl[(o, sb)],
                                w_tiles[o][:, c * 128:(c + 1) * 128],
                                xt_sb[:, c * S + sb * 512:
                                      c * S + sb * 512 + 512],
                                start=(c == 0), stop=(c == NCT - 1))
                for o in os_:
                    for sb in sbs:
                        if o < NO:
                            qt = qts.tile([128, 512], BF16, tag="qt",
                                          name=f"qt_{o}_{sb}")
                            nc.scalar.activation(
                                out=qt, in_=psl[(o, sb)],
                                func=mybir.ActivationFunctionType.Copy)
                            pending.append(
                                lambda sb=sb, o=o, qt=qt: emit_rope(sb, o, qt))
                        else:
                            nc.scalar.activation(
                                out=vt_sb[:, sb * 512:(sb + 1) * 512],
                                in_=psl[(o, sb)],
                                func=mybir.ActivationFunctionType.Copy)
                            for t in range(4 * sb, 4 * sb + 4):
                                pending.append(
                                    lambda t=t: emit_vtrans(t))
            drain_pending(len(pending))

        # ------- phase 2+3: attention with interleaved output proj -------
        # Softmax denominators: exp tiles accumulate elementwise in bf16
        # on the vector engine; a single all-ones matmul per (block,head)
        # produces the partition-sum broadcast. The PE cycles freed from
        # per-k-tile denominator matmuls are filled by draining output-
        # projection matmuls (queued once a head group's attention
        # completes) behind each attnV, so the scalar engine's exp chain
        # never starves the PE. gpsimd stays idle (power throttle).
        aotp = outer.enter_context(tc.tile_pool(name="aot", bufs=1))
        aot = aotp.tile([128, NJT * S], BF16)
        wotp = outer.enter_context(tc.tile_pool(name="wotsb", bufs=1))
        wot_sb = wotp.tile([128, NJT * D], BF16)
        # wot streams on the scalar queue while attention's first block
        # runs; phase 3 first reads it ~12us later.
        for j in range(NJT):
            nc.scalar.dma_start(out=wot_sb[:, j * D:(j + 1) * D],
                                in_=wot_d[j])

        from collections import deque

        with ExitStack() as ph2:
            etp = ph2.enter_context(tc.tile_pool(name="expt", bufs=6))
            accp = ph2.enter_context(tc.tile_pool(name="accf", bufs=2))
            rbp = ph2.enter_context(tc.tile_pool(name="rbc", bufs=2))
            stgp = ph2.enter_context(tc.tile_pool(name="stage", bufs=2))
            spsp = ph2.enter_context(tc.tile_pool(name="sps", bufs=3,
                                                  space="PSUM"))
            outpp = ph2.enter_context(tc.tile_pool(name="outps", bufs=1,
                                                   space="PSUM"))
            opsp = ph2.enter_context(tc.tile_pool(name="ops", bufs=4,
                                                  space="PSUM"))

            ph3q = deque()

            def push_ph3(stc):
                stage = stgp.tile([128, D], BF16, tag="stage",
                                  name=f"stage_{stc}")
                for half in range(2):
                    psl = [opsp.tile([128, 512], F32, tag="ops",
                                     name=f"ops_{stc}_{half}_{i}")
                           for i in range(4)]
                    for j in range(NJT):
                        for i in range(4):
                            eb = half * 4 + i
                            def mm(ps=psl[i], j=j, eb=eb, stc=stc):
                                nc.tensor.matmul(
                                    ps,
                                    aot[:, j * S + stc * 128:
                                        j * S + (stc + 1) * 128],
                                    wot_sb[:, j * D + eb * 512:
                                           j * D + (eb + 1) * 512],
                                    start=(j == 0), stop=(j == NJT - 1))
                            ph3q.append((1, mm))
                            if j == NJT - 1:
                                def ev(ps=psl[i], eb=eb, stage=stage):
                                    dst = stage[:, eb * 512:(eb + 1) * 512]
                                    if eb % 2 == 0:
                                        nc.scalar.activation(
                                            out=dst, in_=ps,
                                            func=mybir.ActivationFunctionType.Copy)
                                    else:
                                        nc.vector.tensor_copy(dst, ps)
                                ph3q.append((0, ev))

                def st(stc=stc, stage=stage):
                    r0, r1 = stc * 128, (stc + 1) * 128
                    if stc == NST - 1:
                        nc.sync.dma_start(out=out_d[r0:r1, :D // 2],
                                          in_=stage[:, :D // 2])
                        nc.scalar.dma_start(out=out_d[r0:r1, D // 2:],
                                            in_=stage[:, D // 2:])
                    else:
                        nc.sync.dma_start(out=out_d[r0:r1, :], in_=stage)
                ph3q.append((0, st))

            def drain_ph3(budget):
                while ph3q and (budget is None or budget > 0
                                or ph3q[0][0] == 0):
                    cost, fn = ph3q.popleft()
                    fn()
                    if budget is not None:
                        budget -= cost

            DRAIN_K = {0: 0, 1: 4, 2: 3, 3: 2}

            for jq in range(NSB):
                nk = 4 * jq + 4       # causal: k-tiles 0..4jq+3
                kdrain = DRAIN_K[jq]
                for h in range(HQ):
                    accf = accp.tile([128, 512], BF16, tag="accf")
                    outps = outpp.tile([128, 512], F32, tag="outps")
                    ets = [None] * nk

                    def stage_a(kt):
                        delta = kt - 4 * jq
                        a = max(delta, 0) * 128   # live q range [a, 512)
                        sps = spsp.tile([128, 512], F32, tag="sps")
                        nc.tensor.matmul(
                            sps[:, a:],
                            qk_rot[:, HQ * S + kt * 128:
                                   HQ * S + (kt + 1) * 128],
                            qk_rot[:, h * S + jq * 512 + a:
                                   h * S + jq * 512 + 512],
                            start=True, stop=True)
                        if delta >= 0:
                            nc.vector.tensor_add(
                                sps[:, a:a + 128], sps[:, a:a + 128],
                                mask_sb)
                        et = etp.tile([128, 512], BF16, tag="et")
                        nc.scalar.activation(
                            out=et[:, a:], in_=sps[:, a:],
                            func=mybir.ActivationFunctionType.Exp,
                            scale=SCALE)
                        ets[kt] = (et, a)

                    def stage_b(kt):
                        et, a = ets[kt]
                        nc.tensor.matmul(
                            outps[:, a:], v_nat[:, kt * 128:(kt + 1) * 128],
                            et[:, a:],
                            start=(kt == 0), stop=(kt == nk - 1))
                        if kt == 0:
                            nc.vector.tensor_copy(accf, et)
                        else:
                            nc.vector.tensor_add(accf[:, a:], accf[:, a:],
                                                 et[:, a:])
                        drain_ph3(kdrain)

                    # depth-2 software pipeline: scores/exp run one k-tile
                    # ahead of attnV so the PE never waits on the exp.
                    for kt in range(nk):
                        stage_a(kt)
                        if kt >= 1:
                            stage_b(kt - 1)
                    stage_b(nk - 1)

                    rps = spsp.tile([128, 512], F32, tag="sps")
                    nc.tensor.matmul(rps, ones_sb, accf, start=True,
                                     stop=True)
                    rinv = rbp.tile([128, 512], F32, tag="rinv")
                    nc.vector.reciprocal_approx_fast(out=rinv, in_=rps)
                    nc.vector.tensor_mul(
                        aot[:, h * S + jq * 512: h * S + jq * 512 + 512],
                        outps, rinv)
                    drain_ph3(2)
                for stc in range(4 * jq, 4 * jq + 4):
                    push_ph3(stc)
                if jq == NSB - 1:
                    drain_ph3(None)

    nc.compile()
    return nc


# ---------------------------------------------------------------------------
# host-side prep


def make_consts(cos, sin):
    """cos/sin: [S, 64] f32 -> replicated T-layout + J + identity + diag mask."""
    cos2 = np.repeat(np.ascontiguousarray(cos.T), 2, axis=0).astype(np.float32)
    sin2 = np.repeat(np.ascontiguousarray(sin.T), 2, axis=0).astype(np.float32)
    J = np.zeros((128, 128), np.float32)
    for p in range(64):
        J[2 * p, 2 * p + 1] = -1.0
        J[2 * p + 1, 2 * p] = 1.0
    jt = np.ascontiguousarray(J.T).astype(NBF)
    ident = np.eye(128, dtype=NBF)
    k_idx = np.arange(128)[:, None]
    q_idx = np.arange(128)[None, :]
    maskt = np.where(q_idx >= k_idx, 0.0, NEG).astype(np.float32)  # [k, q]
    return cos2, sin2, jt, ident, maskt.astype(NBF)


def prep_all(x, wq, wk, wv, wo, cos, sin, n_cores=N_CORES):
    x2 = np.asarray(x, np.float32).reshape(S, D)
    xt = np.ascontiguousarray(x2.T).astype(NBF)
    wq = np.asarray(wq, np.float32)
    wk = np.asarray(wk, np.float32)
    wv = np.asarray(wv, np.float32)
    wo = np.asarray(wo, np.float32)
    cos2, sin2, jt, ident, maskt = make_consts(
        np.asarray(cos, np.float32), np.asarray(sin, np.float32))
    in_maps = []
    for g in range(n_cores):
        w_cat = np.concatenate(
            [wq[g * 512:(g + 1) * 512],
             wk[g * 128:(g + 1) * 128],
             wv[g * 128:(g + 1) * 128]], axis=0)          # [768, D]
        # wt[o, p, c*128 + f] = w_cat[o*128 + f, c*128 + p]
        wt = np.ascontiguousarray(
            w_cat.reshape(NOV, 128, NCT, 128).transpose(0, 3, 2, 1)
        ).reshape(NOV, 128, NCT * 128).astype(NBF)
        wot = np.ascontiguousarray(
            wo[:, g * 512:(g + 1) * 512].T).reshape(4, 128, D).astype(NBF)
        in_maps.append({
            "xt": xt, "wt": wt, "wot": wot, "cos2": cos2, "sin2": sin2,
            "jt": jt, "ident": ident, "maskt": maskt,
        })
    return in_maps


_NC_CACHE = None


def _get_nc():
    global _NC_CACHE
    if _NC_CACHE is None:
        _NC_CACHE = build_nc()
    return _NC_CACHE


def kernel(x, wq, wk, wv, wo, cos, sin, mask, start_pos):
    # mask is the standard causal mask (start_pos=0 prefill) — the kernel
    # applies causality structurally, so neither input is shipped.
    from concourse.bass_utils import run_bass_kernel_spmd

    nc = _get_nc()
    in_maps = prep_all(x, wq, wk, wv, wo, cos, sin)
    res = run_bass_kernel_spmd(nc, in_maps, core_ids=list(range(N_CORES)))
    acc = np.zeros((S, D), np.float32)
    for r in res.results:
        acc += r["out"].astype(np.float32)
    return acc.reshape(1, S, D)


# revision 22
# speedup vs baseline: 1.4934x; 1.0118x over previous
"""Tensor-parallel GQA attention prefill (B=1, S=2048, D=4096, 32 q-heads /
8 kv-heads, RoPE, causal) for 8 Trainium2 NeuronCores.

Sharding: head-parallel. Core g owns q-heads 4g..4g+3 and kv-head g
(exact GQA group), computes Q/K/V projections for its heads, RoPE,
causal attention, and the partial output projection over its 512
contraction dims of wo. The host sums the 8 partial outputs.

Per-core kernel (Bass/Tile), v3:

  phase 1  Q/K/V projections from a resident transposed activation.
           x streams in as 64 half-row [128,1024] transfers on the sync
           queue; the first two o-tiles are computed interleaved at
           seq-half granularity so the PE's consumption rate matches the
           x arrival rate (no startup stall), then o=2..5 run o-major
           with 4-seq-block stationary reuse. Weights and the output
           projection weight stream on the scalar queue. RoPE runs in
           reserved slots of the following pass.
  phase 2  attention transposed (scoresT per k-tile, scalar-engine exp,
           attnV + all-ones denominator matmuls accumulating in PSUM —
           the denominator matmuls double as PE filler behind the exp
           chain). Normalization = approx-reciprocal + multiply on DVE.
  phase 3  output projection per 128-row chunk over 8 PSUM banks;
           evictions alternate scalar/vector, each chunk stored with a
           single 1MB DMA (the last chunk split across two queues so the
           kernel tail is short).

All matmuls run in bf16 with fp32 PSUM accumulation.
"""

import sys

if "/opt/trn_rl_repo" not in sys.path:
    sys.path.insert(0, "/opt/trn_rl_repo")

from contextlib import ExitStack

import numpy as np
import ml_dtypes

import concourse.bass as bass
import concourse.tile as tile
from concourse import mybir, bacc

BF16 = mybir.dt.bfloat16
F32 = mybir.dt.float32
NBF = ml_dtypes.bfloat16

S = 2048
D = 4096
HD = 128
HQ = 4                      # q heads per core
N_CORES = 8
SCALE = 1.0 / float(np.sqrt(128.0))
NEG = -1e9

NCT = D // 128              # contraction tiles over model dim
NSB = S // 512              # 512-wide seq blocks
NST = S // 128              # 128-wide seq tiles
NOV = HQ + 2                # o-tiles: 4 q heads + k + v
NO = HQ + 1                 # rotated o-tiles (q heads + k)
NJT = HQ                    # contraction j-tiles in output proj
NEB = D // 512              # output-proj e blocks
WCOLS = NCT * 128           # per-o weight row length


def build_nc(num_devices=N_CORES):
    nc = bacc.Bacc("TRN2", target_bir_lowering=False, debug=False,
                   num_devices=num_devices)
    xt_d = nc.dram_tensor("xt", [D, S], BF16, kind="ExternalInput")
    wt_d = nc.dram_tensor("wt", [NOV, 128, WCOLS], BF16, kind="ExternalInput")
    wot_d = nc.dram_tensor("wot", [NJT, 128, D], BF16, kind="ExternalInput")
    cos2_d = nc.dram_tensor("cos2", [128, S], F32, kind="ExternalInput")
    sin2_d = nc.dram_tensor("sin2", [128, S], F32, kind="ExternalInput")
    jt_d = nc.dram_tensor("jt", [128, 128], BF16, kind="ExternalInput")
    id_d = nc.dram_tensor("ident", [128, 128], BF16, kind="ExternalInput")
    mask_d = nc.dram_tensor("maskt", [128, 128], BF16, kind="ExternalInput")
    out_d = nc.dram_tensor("out", [S, D], BF16, kind="ExternalOutput")

    with tile.TileContext(nc) as tc, ExitStack() as outer:
        const = outer.enter_context(tc.tile_pool(name="const", bufs=1))
        qkp = outer.enter_context(tc.tile_pool(name="qkrot", bufs=1))
        vp = outer.enter_context(tc.tile_pool(name="vnat", bufs=1))

        jt_sb = const.tile([128, 128], BF16)
        id_sb = const.tile([128, 128], BF16)
        mask_sb = const.tile([128, 128], BF16)
        ones_sb = const.tile([128, 128], BF16)

        qk_rot = qkp.tile([128, NO * S], BF16)
        v_nat = vp.tile([128, S], BF16)

        # ---------------- phase 1: projections + RoPE ----------------
        with ExitStack() as ph1:
            xtp = ph1.enter_context(tc.tile_pool(name="xtp", bufs=1))
            csp = ph1.enter_context(tc.tile_pool(name="cossin", bufs=1))
            wst = ph1.enter_context(tc.tile_pool(name="wstream", bufs=3))
            vts = ph1.enter_context(tc.tile_pool(name="vtsb", bufs=1))
            qts = ph1.enter_context(tc.tile_pool(name="qtmp", bufs=5))
            rtm = ph1.enter_context(tc.tile_pool(name="ropetmp", bufs=2))
            pps = ph1.enter_context(tc.tile_pool(name="projps", bufs=8,
                                                 space="PSUM"))

            xt_sb = xtp.tile([128, NCT * S], BF16)
            cos_sb = csp.tile([128, S], F32)
            sin_sb = csp.tile([128, S], F32)
            vt_sb = vts.tile([128, S], BF16)

            # sync queue: consts, then x half-rows (all first halves then
            # all second halves — the opening passes read only firsts).
            nc.sync.dma_start(out=jt_sb, in_=jt_d[:])
            nc.sync.dma_start(out=id_sb, in_=id_d[:])
            nc.sync.dma_start(out=mask_sb, in_=mask_d[:])
            hS = S // 2
            for c in range(NCT):
                nc.sync.dma_start(out=xt_sb[:, c * S: c * S + hS],
                                  in_=xt_d[c * 128:(c + 1) * 128, :hS])
            for c in range(NCT):
                nc.sync.dma_start(out=xt_sb[:, c * S + hS:(c + 1) * S],
                                  in_=xt_d[c * 128:(c + 1) * 128, hS:])
            nc.vector.memset(ones_sb, 1.0)

            # w tiles rotate through 3 bufs; o>=3 are allocated lazily
            # (after the pass that frees their buffer has been emitted)
            # and ride the sync queue where their buffer-reuse wait
            # blocks nothing that is still needed.
            w_tiles = {}

            def load_w(o, chunks=1):
                w = wst.tile([128, WCOLS], BF16, tag="wsb", name=f"wsb_{o}")
                qn = WCOLS // chunks
                for qd in range(chunks):
                    eng = nc.scalar if o < 2 else nc.sync
                    eng.dma_start(out=w[:, qd * qn:(qd + 1) * qn],
                                  in_=wt_d[o, :, qd * qn:(qd + 1) * qn])
                w_tiles[o] = w

            # scalar queue: the opening passes' weights (o=0/o=1 chunks
            # interleaved so both stream in from c=0), then cos/sin.
            w0 = wst.tile([128, WCOLS], BF16, tag="wsb", name="wsb_0")
            w1 = wst.tile([128, WCOLS], BF16, tag="wsb", name="wsb_1")
            w_tiles[0], w_tiles[1] = w0, w1
            qn = WCOLS // 8
            for qd in range(8):
                for o in range(2):
                    nc.scalar.dma_start(
                        out=w_tiles[o][:, qd * qn:(qd + 1) * qn],
                        in_=wt_d[o, :, qd * qn:(qd + 1) * qn])
            nc.scalar.dma_start(out=cos_sb, in_=cos2_d[:])
            nc.scalar.dma_start(out=sin_sb, in_=sin2_d[:])
            load_w(2)

            def emit_rope(sb, o, qt):
                jp = pps.tile([128, 512], F32, tag="ps", name=f"jp_{sb}_{o}")
                nc.tensor.matmul(jp, jt_sb, qt, start=True, stop=True)
                t1 = rtm.tile([128, 512], F32, tag="rt")
                nc.vector.tensor_mul(
                    t1, qt, cos_sb[:, sb * 512:(sb + 1) * 512])
                nc.vector.tensor_mul(
                    jp, jp, sin_sb[:, sb * 512:(sb + 1) * 512])
                nc.vector.tensor_add(
                    qk_rot[:, o * S + sb * 512: o * S + sb * 512 + 512],
                    t1, jp)

            def emit_vtrans(t):
                tp = pps.tile([128, 128], BF16, tag="ps", name=f"tp_{t}")
                nc.tensor.transpose(tp, vt_sb[:, t * 128:(t + 1) * 128],
                                    id_sb)
                nc.vector.tensor_copy(v_nat[:, t * 128:(t + 1) * 128], tp)

            # pass list: (o list, sb list); the first two passes cover
            # o=0,1 at seq-half granularity so PE consumption (~0.86us/c)
            # tracks x arrival (~0.73us/c half-rows).
            passes = [([0, 1], [0, 1]), ([0, 1], [2, 3])]
            passes += [([o], [0, 1, 2, 3]) for o in range(2, NOV)]

            # rope/vtrans work generated by earlier passes, consumed in
            # reserved slots of later passes' c-loops.
            pending = []

            def drain_pending(n):
                while pending and n > 0:
                    pending.pop(0)()
                    n -= 1

            for pi, (os_, sbs) in enumerate(passes):
                psl = {}
                for o in os_:
                    for sb in sbs:
                        psl[(o, sb)] = pps.tile([128, 512], F32, tag="ps",
                                                name=f"ps_{o}_{sb}")
                for c in range(NCT):
                    if c >= 2:
                        drain_pending(1)
                    for o in os_:
                        for sb in sbs:
                            nc.tensor.matmul(
                                psl[(o, sb)],
                                w_tiles[o][:, c * 128:(c + 1) * 128],
                                xt_sb[:, c * S + sb * 512:
                                      c * S + sb * 512 + 512],
                                start=(c == 0), stop=(c == NCT - 1))
                if 1 <= pi <= 3:
                    load_w(pi + 2)
                for o in os_:
                    for sb in sbs:
                        if o < NO:
                            qt = qts.tile([128, 512], BF16, tag="qt",
                                          name=f"qt_{o}_{sb}")
                            nc.scalar.activation(
                                out=qt, in_=psl[(o, sb)],
                                func=mybir.ActivationFunctionType.Copy)
                            pending.append(
                                lambda sb=sb, o=o, qt=qt: emit_rope(sb, o, qt))
                        else:
                            nc.scalar.activation(
                                out=vt_sb[:, sb * 512:(sb + 1) * 512],
                                in_=psl[(o, sb)],
                                func=mybir.ActivationFunctionType.Copy)
                            for t in range(4 * sb, 4 * sb + 4):
                                pending.append(
                                    lambda t=t: emit_vtrans(t))
            drain_pending(len(pending))

        # ------- phase 2+3: attention with interleaved output proj -------
        # Softmax denominators: exp tiles accumulate elementwise in bf16
        # on the vector engine; a single all-ones matmul per (block,head)
        # produces the partition-sum broadcast. The PE cycles freed from
        # per-k-tile denominator matmuls are filled by draining output-
        # projection matmuls (queued once a head group's attention
        # completes) behind each attnV, so the scalar engine's exp chain
        # never starves the PE. gpsimd stays idle (power throttle).
        aotp = outer.enter_context(tc.tile_pool(name="aot", bufs=1))
        aot = aotp.tile([128, NJT * S], BF16)
        wotp = outer.enter_context(tc.tile_pool(name="wotsb", bufs=1))
        wot_sb = wotp.tile([128, NJT * D], BF16)
        # wot streams on the scalar queue while attention's first block
        # runs; phase 3 first reads it ~12us later.
        for j in range(NJT):
            nc.scalar.dma_start(out=wot_sb[:, j * D:(j + 1) * D],
                                in_=wot_d[j])

        from collections import deque

        with ExitStack() as ph2:
            etp = ph2.enter_context(tc.tile_pool(name="expt", bufs=6))
            accp = ph2.enter_context(tc.tile_pool(name="accf", bufs=2))
            rbp = ph2.enter_context(tc.tile_pool(name="rbc", bufs=2))
            stgp = ph2.enter_context(tc.tile_pool(name="stage", bufs=2))
            spsp = ph2.enter_context(tc.tile_pool(name="sps", bufs=3,
                                                  space="PSUM"))
            outpp = ph2.enter_context(tc.tile_pool(name="outps", bufs=1,
                                                   space="PSUM"))
            opsp = ph2.enter_context(tc.tile_pool(name="ops", bufs=4,
                                                  space="PSUM"))

            ph3q = deque()

            def push_ph3(stc):
                stage = stgp.tile([128, D], BF16, tag="stage",
                                  name=f"stage_{stc}")
                for half in range(2):
                    psl = [opsp.tile([128, 512], F32, tag="ops",
                                     name=f"ops_{stc}_{half}_{i}")
                           for i in range(4)]
                    for j in range(NJT):
                        for i in range(4):
                            eb = half * 4 + i
                            def mm(ps=psl[i], j=j, eb=eb, stc=stc):
                                nc.tensor.matmul(
                                    ps,
                                    aot[:, j * S + stc * 128:
                                        j * S + (stc + 1) * 128],
                                    wot_sb[:, j * D + eb * 512:
                                           j * D + (eb + 1) * 512],
                                    start=(j == 0), stop=(j == NJT - 1))
                            ph3q.append((1, mm))
                            if j == NJT - 1:
                                def ev(ps=psl[i], eb=eb, stage=stage):
                                    dst = stage[:, eb * 512:(eb + 1) * 512]
                                    if eb % 2 == 0:
                                        nc.scalar.activation(
                                            out=dst, in_=ps,
                                            func=mybir.ActivationFunctionType.Copy)
                                    else:
                                        nc.vector.tensor_copy(dst, ps)
                                ph3q.append((0, ev))

                def st(stc=stc, stage=stage):
                    r0, r1 = stc * 128, (stc + 1) * 128
                    if stc == NST - 1:
                        nc.sync.dma_start(out=out_d[r0:r1, :D // 2],
                                          in_=stage[:, :D // 2])
                        nc.scalar.dma_start(out=out_d[r0:r1, D // 2:],
                                            in_=stage[:, D // 2:])
                    else:
                        nc.sync.dma_start(out=out_d[r0:r1, :], in_=stage)
                ph3q.append((0, st))

            def drain_ph3(budget):
                while ph3q and (budget is None or budget > 0
                                or ph3q[0][0] == 0):
                    cost, fn = ph3q.popleft()
                    fn()
                    if budget is not None:
                        budget -= cost

            DRAIN_K = {0: 0, 1: 4, 2: 3, 3: 2}

            for jq in range(NSB):
                nk = 4 * jq + 4       # causal: k-tiles 0..4jq+3
                kdrain = DRAIN_K[jq]
                for h in range(HQ):
                    accf = accp.tile([128, 512], BF16, tag="accf")
                    outps = outpp.tile([128, 512], F32, tag="outps")
                    ets = [None] * nk

                    def stage_a(kt):
                        delta = kt - 4 * jq
                        a = max(delta, 0) * 128   # live q range [a, 512)
                        sps = spsp.tile([128, 512], F32, tag="sps")
                        nc.tensor.matmul(
                            sps[:, a:],
                            qk_rot[:, HQ * S + kt * 128:
                                   HQ * S + (kt + 1) * 128],
                            qk_rot[:, h * S + jq * 512 + a:
                                   h * S + jq * 512 + 512],
                            start=True, stop=True)
                        if delta >= 0:
                            nc.vector.tensor_add(
                                sps[:, a:a + 128], sps[:, a:a + 128],
                                mask_sb)
                        et = etp.tile([128, 512], BF16, tag="et")
                        nc.scalar.activation(
                            out=et[:, a:], in_=sps[:, a:],
                            func=mybir.ActivationFunctionType.Exp,
                            scale=SCALE)
                        ets[kt] = (et, a)

                    def stage_b(kt):
                        et, a = ets[kt]
                        nc.tensor.matmul(
                            outps[:, a:], v_nat[:, kt * 128:(kt + 1) * 128],
                            et[:, a:],
                            start=(kt == 0), stop=(kt == nk - 1))
                        if kt == 0:
                            nc.vector.tensor_copy(accf, et)
                        else:
                            nc.vector.tensor_add(accf[:, a:], accf[:, a:],
                                                 et[:, a:])
                        drain_ph3(kdrain)

                    # depth-2 software pipeline: scores/exp run one k-tile
                    # ahead of attnV so the PE never waits on the exp.
                    for kt in range(nk):
                        stage_a(kt)
                        if kt >= 1:
                            stage_b(kt - 1)
                    stage_b(nk - 1)

                    rps = spsp.tile([128, 512], F32, tag="sps")
                    nc.tensor.matmul(rps, ones_sb, accf, start=True,
                                     stop=True)
                    rinv = rbp.tile([128, 512], F32, tag="rinv")
                    nc.vector.reciprocal_approx_fast(out=rinv, in_=rps)
                    nc.vector.tensor_mul(
                        aot[:, h * S + jq * 512: h * S + jq * 512 + 512],
                        outps, rinv)
                    drain_ph3(2)
                for stc in range(4 * jq, 4 * jq + 4):
                    push_ph3(stc)
                if jq == NSB - 1:
                    drain_ph3(None)

    nc.compile()
    return nc


# ---------------------------------------------------------------------------
# host-side prep


def make_consts(cos, sin):
    """cos/sin: [S, 64] f32 -> replicated T-layout + J + identity + diag mask."""
    cos2 = np.repeat(np.ascontiguousarray(cos.T), 2, axis=0).astype(np.float32)
    sin2 = np.repeat(np.ascontiguousarray(sin.T), 2, axis=0).astype(np.float32)
    J = np.zeros((128, 128), np.float32)
    for p in range(64):
        J[2 * p, 2 * p + 1] = -1.0
        J[2 * p + 1, 2 * p] = 1.0
    jt = np.ascontiguousarray(J.T).astype(NBF)
    ident = np.eye(128, dtype=NBF)
    k_idx = np.arange(128)[:, None]
    q_idx = np.arange(128)[None, :]
    maskt = np.where(q_idx >= k_idx, 0.0, NEG).astype(np.float32)  # [k, q]
    return cos2, sin2, jt, ident, maskt.astype(NBF)


def prep_all(x, wq, wk, wv, wo, cos, sin, n_cores=N_CORES):
    x2 = np.asarray(x, np.float32).reshape(S, D)
    xt = np.ascontiguousarray(x2.T).astype(NBF)
    wq = np.asarray(wq, np.float32)
    wk = np.asarray(wk, np.float32)
    wv = np.asarray(wv, np.float32)
    wo = np.asarray(wo, np.float32)
    cos2, sin2, jt, ident, maskt = make_consts(
        np.asarray(cos, np.float32), np.asarray(sin, np.float32))
    in_maps = []
    for g in range(n_cores):
        w_cat = np.concatenate(
            [wq[g * 512:(g + 1) * 512],
             wk[g * 128:(g + 1) * 128],
             wv[g * 128:(g + 1) * 128]], axis=0)          # [768, D]
        # wt[o, p, c*128 + f] = w_cat[o*128 + f, c*128 + p]
        wt = np.ascontiguousarray(
            w_cat.reshape(NOV, 128, NCT, 128).transpose(0, 3, 2, 1)
        ).reshape(NOV, 128, NCT * 128).astype(NBF)
        wot = np.ascontiguousarray(
            wo[:, g * 512:(g + 1) * 512].T).reshape(4, 128, D).astype(NBF)
        in_maps.append({
            "xt": xt, "wt": wt, "wot": wot, "cos2": cos2, "sin2": sin2,
            "jt": jt, "ident": ident, "maskt": maskt,
        })
    return in_maps


_NC_CACHE = None


def _get_nc():
    global _NC_CACHE
    if _NC_CACHE is None:
        _NC_CACHE = build_nc()
    return _NC_CACHE


def kernel(x, wq, wk, wv, wo, cos, sin, mask, start_pos):
    # mask is the standard causal mask (start_pos=0 prefill) — the kernel
    # applies causality structurally, so neither input is shipped.
    from concourse.bass_utils import run_bass_kernel_spmd

    nc = _get_nc()
    in_maps = prep_all(x, wq, wk, wv, wo, cos, sin)
    res = run_bass_kernel_spmd(nc, in_maps, core_ids=list(range(N_CORES)))
    acc = np.zeros((S, D), np.float32)
    for r in res.results:
        acc += r["out"].astype(np.float32)
    return acc.reshape(1, S, D)
